# revision 4
# baseline (speedup 1.0000x reference)
"""Multi-head causal attention (B=2, S=2048, D=1024, H=16, dh=64) on 8 TRN2 cores.

Strategy
--------
- Shard the 32 (batch, head) pairs across 8 cores, 4 pairs each (cores 0-3: b=0,
  cores 4-7: b=1). Pure data parallel, no collectives.
- Per head, compute S^T = K @ Q^T directly on the PE (contraction over dh=64 on
  the partition axis), so softmax-exp output P^T = exp(S^T) is already in the
  [k, q] layout the P@V matmul needs as lhsT/rhs -- no on-device transposes.
- Softmax without max-subtraction (scores are O(1) after the 1/sqrt(dh) scale,
  exp never overflows in fp32; identical result up to fp rounding).
- Row sums l_q come for free from the P@V matmul by appending a ones-column to
  V ([2048, 65]); output row 64 of O^T accumulates sum_k P^T[k, q].
- Normalization (divide by l) and the final [65, S] -> [S, 64] transpose happen
  on the host, so the device writes O^T straight from PSUM.
- Two heads are packed per 128 SBUF partitions; their K=64-contraction S^T
  matmuls issue to disjoint PE row-groups (tile_position auto-derived from the
  base partition) and run concurrently on the systolic array.
- The mask is handled by host-side block planning at [128 k x 512 q]
  granularity: all-masked blocks are skipped, fully-kept blocks run unmasked,
  and mixed blocks get a 0/1 multiply from a small set of deduplicated mask
  tiles uploaded per core. For the causal mask this is exactly flash-style
  block skipping (~2x work saving) with 4 unique diagonal tiles.
- All matmuls use float32r: measured on HW at fp32-level accuracy (rel err
  1.6e-4 vs f64, identical to the fp32 path) at 4x the fp32 matmul rate.
"""

import os
import sys
from contextlib import ExitStack

import numpy as np

for _p in ("/opt/trn_rl_repo", "/root/.axon_site/_ro/trn_rl_repo"):
    if os.path.isdir(_p) and _p not in sys.path:
        sys.path.insert(0, _p)
        break

import concourse.bass as bass  # noqa: E402
import concourse.bacc as bacc  # noqa: E402
import concourse.mybir as mybir  # noqa: E402
import concourse.tile as tile  # noqa: E402
from concourse.bass_utils import run_bass_kernel_spmd  # noqa: E402

F32 = mybir.dt.float32
F32R = mybir.dt.float32r
EXP = mybir.ActivationFunctionType.Exp

N_CORES = 8
H = 16
DH = 64
QBLK = 512
KBLK = 128

LAST_RESULTS = None  # BassKernelResults of the most recent kernel() call


def _plan_blocks(mask):
    """Classify [KBLK x QBLK] blocks of S^T per q-chunk, union over batch.

    Returns (plans, uniq_contents):
      plans[qc] = list of (kk, c0, c1, m0, m1, uid); block covers k rows
        kk*KBLK..+KBLK and q columns qc*QBLK+c0..qc*QBLK+c1. If uid >= 0,
        multiply P^T block columns [m0, m1) by mask tile `uid`.
      uniq_contents[uid] = float32 [B, KBLK, QBLK] 0/1 tile (per-batch content).
    The first block of each plan covers the whole column union so its matmul
    can own start=True for the PSUM accumulation group.
    """
    B, S, _ = mask.shape
    NQ, NK = S // QBLK, S // KBLK
    uniq_keys = {}
    uniq_contents = []
    plans = []
    for qc in range(NQ):
        raw = []
        for kk in range(NK):
            sub = mask[:, qc * QBLK:(qc + 1) * QBLK, kk * KBLK:(kk + 1) * KBLK]
            anyk = sub.any(axis=(0, 2))  # [QBLK] column needed?
            if not anyk.any():
                continue
            c0 = int(anyk.argmax())
            c1 = QBLK - int(anyk[::-1].argmax())
            raw.append([kk, c0, c1])
        if not raw:
            plans.append([])
            continue
        C0 = min(b[1] for b in raw)
        C1 = max(b[2] for b in raw)
        fi = next((i for i, b in enumerate(raw) if b[1] == C0 and b[2] == C1),
                  None)
        if fi is None:
            raw[0][1], raw[0][2] = C0, C1  # extend block 0 to cover the union
            fi = 0
        raw.insert(0, raw.pop(fi))
        out = []
        for kk, c0, c1 in raw:
            sub = mask[:, qc * QBLK:(qc + 1) * QBLK, kk * KBLK:(kk + 1) * KBLK]
            allk = sub.all(axis=(0, 2))
            dirty = ~allk
            dirty[:c0] = False
            dirty[c1:] = False
            if dirty.any():
                m0 = int(dirty.argmax())
                m1 = QBLK - int(dirty[::-1].argmax())
                content = np.ones((B, KBLK, QBLK), np.float32)
                for bb in range(B):
                    content[bb, :, m0:m1] = sub[bb, m0:m1, :].T
                key = content.tobytes()
                uid = uniq_keys.get(key)
                if uid is None:
                    uid = len(uniq_contents)
                    uniq_keys[key] = uid
                    uniq_contents.append(content)
            else:
                m0 = m1 = 0
                uid = -1
            out.append((kk, c0, c1, m0, m1, uid))
        plans.append(out)
    return plans, uniq_contents


def _build(S, n_groups, n_pairs, plans, n_uniq):
    """Build the single SPMD program run identically on all cores."""
    NQ, NK = S // QBLK, S // KBLK
    VW = DH + 1  # V with ones column
    nc = bacc.Bacc("TRN2", target_bir_lowering=False, debug=False)
    qt = nc.declare_dram_parameter("qt", [n_groups, 128, S], F32R, isOutput=False)
    kt = nc.declare_dram_parameter("kt", [n_groups, 128, S], F32R, isOutput=False)
    vv = nc.declare_dram_parameter("vv", [n_pairs, 128, NK * VW], F32R,
                                   isOutput=False)
    mk = nc.declare_dram_parameter("mk", [max(n_uniq, 1), 128, QBLK], F32R,
                                   isOutput=False)
    ot = nc.declare_dram_parameter("ot", [n_pairs, VW, S], F32, isOutput=True)

    with tile.TileContext(nc) as tc, ExitStack() as ctx:
        qpool = ctx.enter_context(tc.tile_pool(name="qpool", bufs=2))
        kpool = ctx.enter_context(tc.tile_pool(name="kpool", bufs=2))
        vpool = ctx.enter_context(tc.tile_pool(name="vpool", bufs=3))
        mpool = ctx.enter_context(tc.tile_pool(name="mpool", bufs=1))
        ppool = ctx.enter_context(tc.tile_pool(name="ppool", bufs=4))
        obuf = ctx.enter_context(tc.tile_pool(name="obuf", bufs=4))
        spool = ctx.enter_context(tc.tile_pool(name="spool", bufs=3, space="PSUM"))
        opool = ctx.enter_context(tc.tile_pool(name="opool", bufs=2, space="PSUM"))

        mtile = mpool.tile([128, max(n_uniq, 1) * QBLK], F32R)
        for u in range(n_uniq):
            nc.sync.dma_start(mtile[:, u * QBLK:(u + 1) * QBLK], mk[u])

        for g in range(n_groups):
            ktile = kpool.tile([128, S], F32R, tag="kt")
            nc.sync.dma_start(ktile[:], kt[g])
            qtile = qpool.tile([128, S], F32R, tag="qt")
            nc.sync.dma_start(qtile[:], qt[g])
            vtiles = []
            for h in range(2):
                vt = vpool.tile([128, NK * VW], F32R, tag=f"vt{h}")
                nc.sync.dma_start(vt[:], vv[2 * g + h])
                vtiles.append(vt)

            for qc in range(NQ):
                blocks = plans[qc]
                if not blocks:
                    continue
                nb = len(blocks)
                o_ps = [opool.tile([VW, QBLK], F32, tag=f"o{h}", name=f"o_ps{h}",
                                   bufs=1)
                        for h in range(2)]
                LA = 2  # blocks of PE-lookahead before each P@V accumulate
                staged = []
                for i in range(nb + LA):
                    if i < nb:
                        kk, c0, c1, m0, m1, uid = blocks[i]
                        W = c1 - c0
                        s_ps = spool.tile([128, 2 * QBLK], F32, tag="s")
                        for h in range(2):
                            nc.tensor.matmul(
                                s_ps[:, h * QBLK:h * QBLK + W],
                                lhsT=ktile[64 * h:64 * h + 64,
                                           kk * KBLK:(kk + 1) * KBLK],
                                rhs=qtile[64 * h:64 * h + 64,
                                          qc * QBLK + c0:qc * QBLK + c1],
                                start=True, stop=True)
                        p_t = ppool.tile([128, 2 * QBLK], F32R, tag="p")
                        if W == QBLK:
                            # one pass over both halves (contiguous, no gap)
                            nc.scalar.activation(p_t[:, 0:2 * QBLK],
                                                 s_ps[:, 0:2 * QBLK], EXP)
                        else:
                            for h in range(2):
                                nc.scalar.activation(
                                    p_t[:, h * QBLK:h * QBLK + W],
                                    s_ps[:, h * QBLK:h * QBLK + W], EXP)
                        if uid >= 0:
                            for h in range(2):
                                lo = h * QBLK + (m0 - c0)
                                nc.vector.tensor_mul(
                                    p_t[:, lo:lo + (m1 - m0)],
                                    p_t[:, lo:lo + (m1 - m0)],
                                    mtile[:, uid * QBLK + m0:uid * QBLK + m1])
                        staged.append((i, kk, c0, c1, W, p_t))
                    if i >= LA:
                        j, kk, c0, c1, W, p_t = staged[i - LA]
                        for h in range(2):
                            nc.tensor.matmul(
                                o_ps[h][:, c0:c1],
                                lhsT=vtiles[h][:, kk * VW:(kk + 1) * VW],
                                rhs=p_t[:, h * QBLK:h * QBLK + W],
                                start=(j == 0), stop=(j == nb - 1))
                for h in range(2):
                    osb = obuf.tile([VW, QBLK], F32, tag="osb")
                    nc.vector.tensor_copy(osb[:], o_ps[h][:])
                    nc.sync.dma_start(ot[2 * g + h, :, qc * QBLK:(qc + 1) * QBLK],
                                      osb[:])
    nc.finalize()
    return nc


def _make_in_maps(q4, k4, v4, maskb, uniq, n_groups, per_core):
    B, S = q4.shape[0], q4.shape[1]
    NK = S // KBLK
    VW = DH + 1
    n_uniq = len(uniq)
    in_maps = []
    for c in range(N_CORES):
        qt = np.empty((n_groups, 128, S), np.float32)
        kt = np.empty((n_groups, 128, S), np.float32)
        vvv = np.empty((per_core, 128, NK * VW), np.float32)
        bs = []
        for lp in range(per_core):
            gp = c * per_core + lp
            b, h = divmod(gp, H)
            bs.append(b)
            g, half = divmod(lp, 2)
            qt[g, 64 * half:64 * half + 64] = q4[b, :, h, :].T
            kt[g, 64 * half:64 * half + 64] = k4[b, :, h, :].T
            vt = np.ones((128, NK, VW), np.float32)
            vt[:, :, :DH] = v4[b, :, h, :].reshape(NK, KBLK, DH).transpose(1, 0, 2)
            vvv[lp] = vt.reshape(128, NK * VW)
        if n_uniq:
            assert len(set(bs)) == 1, "mask tiles assume one batch per core"
            mkarr = np.ascontiguousarray(
                np.stack([uniq[u][bs[0]] for u in range(n_uniq)]))
        else:
            mkarr = np.zeros((1, 128, QBLK), np.float32)
        in_maps.append({"qt": qt, "kt": kt, "vv": vvv, "mk": mkarr})
    return in_maps


def _assemble(results, B, S, per_core):
    D = H * DH
    out = np.empty((B, S, D), np.float32)
    for c in range(N_CORES):
        otc = results[c]["ot"]  # [per_core, DH+1, S]
        for lp in range(per_core):
            gp = c * per_core + lp
            b, h = divmod(gp, H)
            l = otc[lp, DH].astype(np.float64)
            l = np.where(l == 0.0, 1.0, l)
            out[b, :, h * DH:(h + 1) * DH] = \
                (otc[lp, :DH] / l).T.astype(np.float32)
    return out


def kernel(queries, keys, values, mask):
    B, S, D = queries.shape
    assert D == H * DH
    q4 = (np.ascontiguousarray(queries, dtype=np.float32) * 0.125) \
        .reshape(B, S, H, DH)
    k4 = np.ascontiguousarray(keys, dtype=np.float32).reshape(B, S, H, DH)
    v4 = np.ascontiguousarray(values, dtype=np.float32).reshape(B, S, H, DH)
    maskb = np.asarray(mask).astype(bool)

    plans, uniq = _plan_blocks(maskb)
    per_core = (B * H) // N_CORES
    n_groups = per_core // 2

    nc = _build(S, n_groups, per_core, plans, len(uniq))
    in_maps = _make_in_maps(q4, k4, v4, maskb, uniq, n_groups, per_core)
    res = run_bass_kernel_spmd(nc, in_maps, core_ids=list(range(N_CORES)))
    global LAST_RESULTS
    LAST_RESULTS = res
    return _assemble(res.results, B, S, per_core)


# revision 18
# speedup vs baseline: 1.0854x; 1.0854x over previous
"""Multi-head causal attention (B=2, S=2048, D=1024, H=16, dh=64) on 8 TRN2 cores.

Strategy
--------
- Shard the 32 (batch, head) pairs across 8 cores, 4 pairs each (cores 0-3: b=0,
  cores 4-7: b=1). Pure data parallel, no collectives.
- Per head, compute S^T = K @ Q^T directly on the PE (contraction over dh=64 on
  the partition axis), so softmax-exp output P^T = exp(S^T) is already in the
  [k, q] layout the P@V matmul needs as lhsT/rhs -- no on-device transposes.
- Softmax without max-subtraction (scores are O(1) after the 1/sqrt(dh) scale,
  exp never overflows in fp32; identical result up to fp rounding).
- Row sums l_q come for free from the P@V matmul by appending a ones-column to
  V ([2048, 65]); output row 64 of O^T accumulates sum_k P^T[k, q].
- Normalization (divide by l) and the final [65, S] -> [S, 64] transpose happen
  on the host, so the device writes O^T straight from PSUM.
- Two heads are packed per 128 SBUF partitions; their K=64-contraction S^T
  matmuls issue to disjoint PE row-groups (tile_position auto-derived from the
  base partition) and run concurrently on the systolic array.
- The mask is handled by host-side block planning at [128 k x 512 q]
  granularity: all-masked blocks are skipped, fully-kept blocks run unmasked,
  and mixed blocks get a 0/1 multiply from a small set of deduplicated mask
  tiles uploaded per core. For the causal mask this is exactly flash-style
  block skipping (~2x work saving) with 4 unique diagonal tiles.
- All matmuls use float32r: measured on HW at fp32-level accuracy (rel err
  1.6e-4 vs f64, identical to the fp32 path) at 4x the fp32 matmul rate.
"""

import os
import sys
from contextlib import ExitStack

import numpy as np

for _p in ("/opt/trn_rl_repo", "/root/.axon_site/_ro/trn_rl_repo"):
    if os.path.isdir(_p) and _p not in sys.path:
        sys.path.insert(0, _p)
        break

import concourse.bass as bass  # noqa: E402
import concourse.bacc as bacc  # noqa: E402
import concourse.mybir as mybir  # noqa: E402
import concourse.tile as tile  # noqa: E402
from concourse.bass_utils import run_bass_kernel_spmd  # noqa: E402

F32 = mybir.dt.float32
F32R = mybir.dt.float32r
EXP = mybir.ActivationFunctionType.Exp

N_CORES = 8
H = 16
DH = 64
QBLK = 512
KBLK = 128

LAST_RESULTS = None  # BassKernelResults of the most recent kernel() call


def _plan_blocks(mask):
    """Classify [KBLK x QBLK] blocks of S^T per q-chunk, union over batch.

    Returns (plans, uniq_contents):
      plans[qc] = list of (kk, c0, c1, m0, m1, uid); block covers k rows
        kk*KBLK..+KBLK and q columns qc*QBLK+c0..qc*QBLK+c1. If uid >= 0,
        multiply P^T block columns [m0, m1) by mask tile `uid`.
      uniq_contents[uid] = float32 [B, KBLK, QBLK] 0/1 tile (per-batch content).
    The first block of each plan covers the whole column union so its matmul
    can own start=True for the PSUM accumulation group.
    """
    B, S, _ = mask.shape
    NQ, NK = S // QBLK, S // KBLK
    uniq_keys = {}
    uniq_contents = []
    plans = []
    for qc in range(NQ):
        raw = []
        for kk in range(NK):
            sub = mask[:, qc * QBLK:(qc + 1) * QBLK, kk * KBLK:(kk + 1) * KBLK]
            anyk = sub.any(axis=(0, 2))  # [QBLK] column needed?
            if not anyk.any():
                continue
            c0 = int(anyk.argmax())
            c1 = QBLK - int(anyk[::-1].argmax())
            raw.append([kk, c0, c1])
        if not raw:
            plans.append([])
            continue
        C0 = min(b[1] for b in raw)
        C1 = max(b[2] for b in raw)
        fi = next((i for i, b in enumerate(raw) if b[1] == C0 and b[2] == C1),
                  None)
        if fi is None:
            raw[0][1], raw[0][2] = C0, C1  # extend block 0 to cover the union
            fi = 0
        raw.insert(0, raw.pop(fi))
        out = []
        for kk, c0, c1 in raw:
            sub = mask[:, qc * QBLK:(qc + 1) * QBLK, kk * KBLK:(kk + 1) * KBLK]
            allk = sub.all(axis=(0, 2))
            dirty = ~allk
            dirty[:c0] = False
            dirty[c1:] = False
            if dirty.any():
                m0 = int(dirty.argmax())
                m1 = QBLK - int(dirty[::-1].argmax())
                content = np.zeros((B, KBLK, m1 - m0), np.float32)
                for bb in range(B):
                    content[bb] = sub[bb, m0:m1, :].T
                key = content.tobytes()
                uid = uniq_keys.get(key)
                if uid is None:
                    uid = len(uniq_contents)
                    uniq_keys[key] = uid
                    uniq_contents.append(content)
            else:
                m0 = m1 = 0
                uid = -1
            out.append((kk, c0, c1, m0, m1, uid))
        plans.append(out)
    mw = max((c.shape[2] for c in uniq_contents), default=1)
    uniq_padded = []
    for c in uniq_contents:
        p = np.zeros((B, KBLK, mw), np.float32)
        p[:, :, :c.shape[2]] = c
        uniq_padded.append(p)
    return plans, uniq_padded


def _build(S, n_groups, n_pairs, plans, n_uniq, mw=1, repeat=1,
           la=2, p_bufs=6, s_bufs=3, o_bufs=1, osb_bufs=4, dma_psum=False):
    """Build the single SPMD program run identically on all cores.

    repeat > 1 re-runs the whole body (for wall-clock benchmarking only).
    """
    NQ, NK = S // QBLK, S // KBLK
    VW = DH + 1  # V with ones column
    nc = bacc.Bacc("TRN2", target_bir_lowering=False, debug=False)
    qt = nc.declare_dram_parameter("qt", [n_groups, 128, S], F32R, isOutput=False)
    kt = nc.declare_dram_parameter("kt", [n_groups, 128, S], F32R, isOutput=False)
    vv = nc.declare_dram_parameter("vv", [n_pairs, 128, NK * VW], F32R,
                                   isOutput=False)
    mk = nc.declare_dram_parameter("mk", [max(n_uniq, 1), 128, mw], F32R,
                                   isOutput=False)
    ot = nc.declare_dram_parameter("ot", [n_pairs, VW, S], F32, isOutput=True)

    with tile.TileContext(nc) as tc, ExitStack() as ctx:
        qpool = ctx.enter_context(tc.tile_pool(name="qpool", bufs=2))
        kpool = ctx.enter_context(tc.tile_pool(name="kpool", bufs=2))
        vpool = ctx.enter_context(tc.tile_pool(name="vpool", bufs=3))
        mpool = ctx.enter_context(tc.tile_pool(name="mpool", bufs=1))
        ppool = ctx.enter_context(tc.tile_pool(name="ppool", bufs=p_bufs))
        obuf = ctx.enter_context(tc.tile_pool(name="obuf", bufs=osb_bufs))
        spool = ctx.enter_context(tc.tile_pool(name="spool", bufs=s_bufs, space="PSUM"))
        opool = ctx.enter_context(tc.tile_pool(name="opool", bufs=2, space="PSUM"))

        # Trigger the ACT exp-table load at t=0 so its ~2.7us overlaps the
        # initial input DMAs instead of delaying the first real exp.
        warm = mpool.tile([128, 8], F32)
        nc.vector.memset(warm[:], 0.0)
        nc.scalar.activation(warm[:], warm[:], EXP)

        mtile = mpool.tile([128, max(n_uniq, 1) * mw], F32R)

        first_group = True
        giter = [g for _ in range(repeat) for g in range(n_groups)]
        for gi, g in enumerate(giter):
            is_last_group = gi == len(giter) - 1
            ktile = kpool.tile([128, S], F32R, tag="kt")
            qtile = qpool.tile([128, S], F32R, tag="qt")
            vtiles = [vpool.tile([128, NK * VW], F32R, tag=f"vt{h}",
                                 name=f"vt{h}") for h in range(2)]
            # chunked loads, first-needed first: the opening S-matmuls only
            # need the leading columns, so don't serialize them behind
            # monolithic 1 MB transfers (DMA is bus-serial at ~330 GB/s)
            nq4 = max(NK // 4, 1) * VW  # V quarter: one qc's worth of kk
            nc.sync.dma_start(ktile[:, 0:KBLK], kt[g, :, 0:KBLK])
            nc.sync.dma_start(qtile[:, 0:QBLK], qt[g, :, 0:QBLK])
            nc.sync.dma_start(ktile[:, KBLK:QBLK], kt[g, :, KBLK:QBLK])
            if first_group:
                for u in range(n_uniq):
                    nc.sync.dma_start(mtile[:, u * mw:(u + 1) * mw], mk[u])
                first_group = False
            for h in range(2):
                nc.sync.dma_start(vtiles[h][:, 0:nq4], vv[2 * g + h, :, 0:nq4])
            vdone = nq4
            for c0 in range(QBLK, S, QBLK):
                nc.sync.dma_start(ktile[:, c0:c0 + QBLK], kt[g, :, c0:c0 + QBLK])
                nc.sync.dma_start(qtile[:, c0:c0 + QBLK], qt[g, :, c0:c0 + QBLK])
                v1 = min(vdone + nq4, NK * VW)
                for h in range(2):
                    if vdone < v1:
                        nc.sync.dma_start(vtiles[h][:, vdone:v1],
                                          vv[2 * g + h, :, vdone:v1])
                vdone = v1
            for h in range(2):
                if vdone < NK * VW:
                    nc.sync.dma_start(vtiles[h][:, vdone:],
                                      vv[2 * g + h, :, vdone:])

            for qc in range(NQ):
                blocks = plans[qc]
                if not blocks:
                    continue
                nb = len(blocks)
                o_ps = [opool.tile([VW, QBLK], F32, tag=f"o{h}", name=f"o_ps{h}",
                                   bufs=o_bufs)
                        for h in range(2)]
                LA = la  # blocks of PE-lookahead before each P@V accumulate
                staged = []
                for i in range(nb + LA):
                    if i < nb:
                        kk, c0, c1, m0, m1, uid = blocks[i]
                        W = c1 - c0
                        # narrow blocks pack both halves into one PSUM bank so
                        # a single (cheaper) exp instruction covers both
                        hoff = QBLK  # BISECT: packing disabled
                        s_ps = spool.tile([128, 2 * QBLK], F32, tag="s")
                        for h in range(2):
                            nc.tensor.matmul(
                                s_ps[:, h * hoff:h * hoff + W],
                                lhsT=ktile[64 * h:64 * h + 64,
                                           kk * KBLK:(kk + 1) * KBLK],
                                rhs=qtile[64 * h:64 * h + 64,
                                          qc * QBLK + c0:qc * QBLK + c1],
                                start=True, stop=True)
                        p_t = ppool.tile([128, 2 * QBLK], F32R, tag="p")
                        if hoff == W:
                            nc.scalar.activation(p_t[:, 0:2 * W],
                                                 s_ps[:, 0:2 * W], EXP)
                        elif W == QBLK:
                            nc.scalar.activation(p_t[:, 0:2 * QBLK],
                                                 s_ps[:, 0:2 * QBLK], EXP)
                        else:
                            for h in range(2):
                                nc.scalar.activation(
                                    p_t[:, h * QBLK:h * QBLK + W],
                                    s_ps[:, h * QBLK:h * QBLK + W], EXP)
                        if uid >= 0:
                            for h in range(2):
                                lo = h * hoff + (m0 - c0)
                                nc.vector.tensor_mul(
                                    p_t[:, lo:lo + (m1 - m0)],
                                    p_t[:, lo:lo + (m1 - m0)],
                                    mtile[:, uid * mw:uid * mw + (m1 - m0)])
                        staged.append((i, kk, c0, c1, W, hoff, p_t))
                    if i >= LA:
                        j, kk, c0, c1, W, hoff, p_t = staged[i - LA]
                        for h in range(2):
                            nc.tensor.matmul(
                                o_ps[h][:, c0:c1],
                                lhsT=vtiles[h][:, kk * VW:(kk + 1) * VW],
                                rhs=p_t[:, h * hoff:h * hoff + W],
                                start=(j == 0), stop=(j == nb - 1))
                for h in range(2):
                    dst = ot[2 * g + h, :, qc * QBLK:(qc + 1) * QBLK]
                    if dma_psum:
                        nc.sync.dma_start(dst, o_ps[h][:])
                    else:
                        osb = obuf.tile([VW, QBLK], F32, tag="osb")
                        nc.vector.tensor_copy(osb[:], o_ps[h][:])
                        if is_last_group and qc == NQ - 1:
                            nc.sync.dma_start(dst, osb[:])
                        else:
                            nc.gpsimd.dma_start(dst, osb[:])
    nc.finalize()
    return nc


def _make_in_maps(q4, k4, v4, maskb, uniq, n_groups, per_core):
    B, S = q4.shape[0], q4.shape[1]
    NK = S // KBLK
    VW = DH + 1
    n_uniq = len(uniq)
    in_maps = []
    for c in range(N_CORES):
        qt = np.empty((n_groups, 128, S), np.float32)
        kt = np.empty((n_groups, 128, S), np.float32)
        vvv = np.empty((per_core, 128, NK * VW), np.float32)
        bs = []
        for lp in range(per_core):
            gp = c * per_core + lp
            b, h = divmod(gp, H)
            bs.append(b)
            g, half = divmod(lp, 2)
            qt[g, 64 * half:64 * half + 64] = q4[b, :, h, :].T
            kt[g, 64 * half:64 * half + 64] = k4[b, :, h, :].T
            vt = np.ones((128, NK, VW), np.float32)
            vt[:, :, :DH] = v4[b, :, h, :].reshape(NK, KBLK, DH).transpose(1, 0, 2)
            vvv[lp] = vt.reshape(128, NK * VW)
        if n_uniq:
            assert len(set(bs)) == 1, "mask tiles assume one batch per core"
            mkarr = np.ascontiguousarray(
                np.stack([uniq[u][bs[0]] for u in range(n_uniq)]))
        else:
            mkarr = np.zeros((1, 128, 1), np.float32)
        in_maps.append({"qt": qt, "kt": kt, "vv": vvv, "mk": mkarr})
    return in_maps


def _assemble(results, B, S, per_core):
    D = H * DH
    out = np.empty((B, S, D), np.float32)
    for c in range(N_CORES):
        otc = results[c]["ot"]  # [per_core, DH+1, S]
        for lp in range(per_core):
            gp = c * per_core + lp
            b, h = divmod(gp, H)
            l = otc[lp, DH].astype(np.float64)
            l = np.where(l == 0.0, 1.0, l)
            out[b, :, h * DH:(h + 1) * DH] = \
                (otc[lp, :DH] / l).T.astype(np.float32)
    return out


def kernel(queries, keys, values, mask):
    B, S, D = queries.shape
    assert D == H * DH
    q4 = (np.ascontiguousarray(queries, dtype=np.float32) * 0.125) \
        .reshape(B, S, H, DH)
    k4 = np.ascontiguousarray(keys, dtype=np.float32).reshape(B, S, H, DH)
    v4 = np.ascontiguousarray(values, dtype=np.float32).reshape(B, S, H, DH)
    maskb = np.asarray(mask).astype(bool)

    plans, uniq = _plan_blocks(maskb)
    per_core = (B * H) // N_CORES
    n_groups = per_core // 2

    mw = uniq[0].shape[2] if uniq else 1
    nc = _build(S, n_groups, per_core, plans, len(uniq), mw=mw)
    in_maps = _make_in_maps(q4, k4, v4, maskb, uniq, n_groups, per_core)
    res = run_bass_kernel_spmd(nc, in_maps, core_ids=list(range(N_CORES)))
    global LAST_RESULTS
    LAST_RESULTS = res
    return _assemble(res.results, B, S, per_core)
